# revision 1
# baseline (speedup 1.0000x reference)
"""Euclidean distance (cdist) kernel for Trainium2, 8 NeuronCores.

out[b, j] = || x[b, :] - weight[:, j] ||_2   for x [4096, 64], weight [64, 50000].

Sharding (per hint): K = 50000 split into 8 slabs of 6250, one per core
(tensor-parallel over prototypes); x replicated; no cross-core reduction.

Math: dist^2 = x2[b] + w2[j] - 2*x@w. The matmul runs in fp32r (the PE's
fast fp32 mode, RNE-rounded to 11 mantissa bits) at 4x the fp32 rate, with
full fp32-level accuracy recovered via a Dekker-style hi/lo split that
exploits the unused contraction capacity (D=64 of 128 partitions):

  mm1: lhsT=[xs_hi; xs_lo] (128 rows) rhs=[w_hi; w_hi]       -> -2x @ w_hi
  mm2: lhsT=[xs_hi; 1; 1]  (66 rows)  rhs=[w_lo; w2_hi; w2_lo]
                                              -> -2x @ w_lo + w2  (accum)
  where xs = -2x, v_hi = rne11(v), v_lo = rne11(v - v_hi).
  PSUM = -2*x'@w' + w2   with x', w' accurate to 22+ mantissa bits.
  ScalarE: out = sqrt(PSUM + x2[b])  (x2 as exact per-partition bias).

All hi/lo operands are rounded on the host (exact emulation of the HW's
fp32r RNE-11 rounding), shipped as float32r DRAM tensors.

Per core: 32 b-tiles of 128 rows; per b-tile 13 j-tiles of <=512 cols
(one PSUM bank); per b-tile a single contiguous 3.2 MB DMA store.
"""

import numpy as np
from contextlib import ExitStack

import concourse.bass as bass
import concourse.bacc as bacc
import concourse.tile as tile
from concourse import mybir
from concourse.bass_utils import run_bass_kernel_spmd

B, D, K = 4096, 64, 50000
NCORES = 8
KS = K // NCORES  # 6250 columns per core
P = 128
JT = 512          # matmul free-dim tile (one PSUM bank of fp32)
D2 = 2 * D        # 128: stacked hi/lo contraction for mm1
DL = D + 2        # 66: contraction for mm2 (w_lo + w2_hi + w2_lo rows)

F32 = mybir.dt.float32
F32R = mybir.dt.float32r


def build_nc(b=B, ks=KS):
    nbt = b // P
    nc = bacc.Bacc("TRN2", target_bir_lowering=False, debug=False)
    xs128 = nc.dram_tensor("xs128", [D2, b], F32R, kind="ExternalInput").ap()
    xs66 = nc.dram_tensor("xs66", [DL, b], F32R, kind="ExternalInput").ap()
    wst1 = nc.dram_tensor("wst1", [D2, ks], F32R, kind="ExternalInput").ap()
    wst2 = nc.dram_tensor("wst2", [DL, ks], F32R, kind="ExternalInput").ap()
    x2 = nc.dram_tensor("x2", [P, nbt], F32, kind="ExternalInput").ap()
    out = nc.dram_tensor("out", [b, ks], F32, kind="ExternalOutput").ap()

    CHUNK = 4 * JT  # 2048: one 4-bank PSUM tile, one ACT instruction
    chunks = [(c0, min(CHUNK, ks - c0)) for c0 in range(0, ks, CHUNK)]

    with tile.TileContext(nc) as tc:
        with ExitStack() as ctx:
            singles = ctx.enter_context(tc.tile_pool(name="singles", bufs=1))
            outp = ctx.enter_context(tc.tile_pool(name="outp", bufs=2))
            psum = ctx.enter_context(tc.tile_pool(name="psum", bufs=2, space="PSUM"))

            # Load order = criticality: the first j-tile's weights + x operands
            # gate the first matmuls; later weight chunks overlap with compute.
            wst1_sb = []
            wst2_sb = []
            for ic, (c0, cn) in enumerate(chunks):
                wst1_sb.append(singles.tile([D2, cn], F32R, name=f"wst1_{c0}"))
                wst2_sb.append(singles.tile([DL, cn], F32R, name=f"wst2_{c0}"))

            # chunk-0 weights and b-tile-0 x slices arrive first so the first
            # matmuls start as early as possible; the bulk follows.
            nc.sync.dma_start(out=wst1_sb[0][:, 0:JT], in_=wst1[:, 0:JT])
            xs128_sb = singles.tile([D2, b], F32R)
            nc.sync.dma_start(out=xs128_sb[:, 0:P], in_=xs128[:, 0:P])
            nc.sync.dma_start(out=wst2_sb[0][:, 0:JT], in_=wst2[:, 0:JT])
            xs66_sb = singles.tile([DL, b], F32R)
            nc.sync.dma_start(out=xs66_sb[:, 0:P], in_=xs66[:, 0:P])
            x2_sb = singles.tile([P, nbt], F32)
            nc.sync.dma_start(out=x2_sb, in_=x2)
            c0n = chunks[0][1]
            nc.sync.dma_start(out=wst1_sb[0][:, JT:c0n], in_=wst1[:, JT:c0n])
            nc.sync.dma_start(out=wst2_sb[0][:, JT:c0n], in_=wst2[:, JT:c0n])
            nc.sync.dma_start(out=xs128_sb[:, P:b], in_=xs128[:, P:b])
            nc.sync.dma_start(out=xs66_sb[:, P:b], in_=xs66[:, P:b])
            for ic, (c0, cn) in enumerate(chunks):
                if ic == 0:
                    continue
                nc.sync.dma_start(out=wst1_sb[ic], in_=wst1[:, c0:c0 + cn])
                nc.sync.dma_start(out=wst2_sb[ic], in_=wst2[:, c0:c0 + cn])

            for ib in range(nbt):
                # Store per chunk only on the first b-tile (starts the store
                # pipeline early); whole-row 3.2 MB stores otherwise — large
                # stores measurably minimize total DMA engine-seconds.
                chunked_store = ib == 0
                ot = outp.tile([P, ks], F32)
                for ic, (c0, cn) in enumerate(chunks):
                    pt = psum.tile([P, CHUNK], F32)
                    for jj in range(0, cn, JT):
                        jn = min(JT, cn - jj)
                        nc.tensor.matmul(
                            pt[:, jj:jj + jn],
                            xs128_sb[:, ib * P:(ib + 1) * P],
                            wst1_sb[ic][:, jj:jj + jn],
                            start=True,
                            stop=False,
                        )
                        nc.tensor.matmul(
                            pt[:, jj:jj + jn],
                            xs66_sb[:, ib * P:(ib + 1) * P],
                            wst2_sb[ic][:, jj:jj + jn],
                            start=False,
                            stop=True,
                        )
                    nc.scalar.activation(
                        ot[:, c0:c0 + cn],
                        pt[:, :cn],
                        mybir.ActivationFunctionType.Sqrt,
                        bias=x2_sb[:, ib:ib + 1],
                        scale=1.0,
                    )
                    if chunked_store:
                        nc.sync.dma_start(
                            out=out[ib * P:(ib + 1) * P, c0:c0 + cn],
                            in_=ot[:, c0:c0 + cn],
                        )
                if not chunked_store:
                    nc.sync.dma_start(out=out[ib * P:(ib + 1) * P, :], in_=ot)
    nc.compile()
    return nc


def _rne11(x):
    """HW-exact fp32r rounding: RNE to 11 mantissa bits."""
    x = np.asarray(x, np.float32)
    u = x.view(np.uint32).astype(np.uint64)
    shift = np.uint64(12)
    half = np.uint64(1 << 11)
    lsb = (u >> shift) & np.uint64(1)
    u2 = (u + half - np.uint64(1) + lsb) >> shift << shift
    return u2.astype(np.uint32).view(np.float32)


def prep_inputs(x, weight):
    """Host-side prep: hi/lo fp32r splits and stacked operand matrices."""
    x = np.ascontiguousarray(x, dtype=np.float32)
    weight = np.ascontiguousarray(weight, dtype=np.float32)
    b, d = x.shape
    k = weight.shape[1]
    x2 = (x.astype(np.float64) ** 2).sum(axis=1).astype(np.float32)
    w2 = (weight.astype(np.float64) ** 2).sum(axis=0).astype(np.float32)

    xs = (-2.0 * x).astype(np.float32)
    xs_hi = _rne11(xs)
    xs_lo = _rne11((xs - xs_hi).astype(np.float32))
    w_hi = _rne11(weight)
    w_lo = _rne11((weight - w_hi).astype(np.float32))
    w2_hi = _rne11(w2)
    w2_lo = _rne11((w2 - w2_hi).astype(np.float32))

    xs128 = np.empty((D2, b), dtype=np.float32)
    xs128[:d] = xs_hi.T
    xs128[d:] = xs_lo.T
    xs66 = np.empty((DL, b), dtype=np.float32)
    xs66[:d] = xs_hi.T
    xs66[d:] = 1.0
    wst1 = np.empty((D2, k), dtype=np.float32)
    wst1[:d] = w_hi
    wst1[d:] = w_hi
    wst2 = np.empty((DL, k), dtype=np.float32)
    wst2[:d] = w_lo
    wst2[d] = w2_hi
    wst2[d + 1] = w2_lo
    x2t = np.ascontiguousarray(x2.reshape(b // P, P).T)  # [P, NBT]
    return xs128, xs66, wst1, wst2, x2t


_nc_cache = {}


def _get_nc():
    if "nc" not in _nc_cache:
        _nc_cache["nc"] = build_nc()
    return _nc_cache["nc"]


def make_in_maps(x, weight, ks=KS):
    xs128, xs66, wst1, wst2, x2t = prep_inputs(x, weight)
    return [
        {"xs128": xs128,
         "xs66": xs66,
         "wst1": np.ascontiguousarray(wst1[:, i * ks:(i + 1) * ks]),
         "wst2": np.ascontiguousarray(wst2[:, i * ks:(i + 1) * ks]),
         "x2": x2t}
        for i in range(NCORES)
    ]


def kernel(x, weight):
    nc = _get_nc()
    in_maps = make_in_maps(x, weight)
    res = run_bass_kernel_spmd(nc, in_maps, core_ids=list(range(NCORES)))
    return np.concatenate([res.results[i]["out"] for i in range(NCORES)], axis=1)



# revision 2
# speedup vs baseline: 1.4525x; 1.4525x over previous
"""Euclidean distance (cdist) kernel for Trainium2, 8 NeuronCores.

out[b, j] = || x[b, :] - weight[:, j] ||_2   for x [4096, 64], weight [64, 50000].

Sharding (per hint): K = 50000 split into 8 slabs of 6250, one per core
(tensor-parallel over prototypes); x replicated; no cross-core reduction.

Math: dist^2 = x2[b] + w2[j] - 2*x@w, folded into ONE matmul with a 68-row
contraction of fp16 operands (fp16 streams at 1 col/cycle on the PE, 2x the
measured fp32r rate, and its 10-bit mantissa matches fp32r accuracy):

  lhsT [68, B]:  rows 0-63 = (-2x).T, 64-65 = 1, 66 = x2_hi, 67 = x2_lo
  rhs  [68, K]:  rows 0-63 = w,  64 = w2_hi, 65 = w2_lo,  66-67 = 1
  PSUM[b, j] = dist^2   (hi/lo fp16 splits keep w2/x2 at 20+ mantissa bits;
                         measured max rel err 5.8e-4 vs fp64 reference)

Output is stored as fp16 (the checker gate is 2e-2 rel; fp16 adds <=4.9e-4),
halving store traffic: 51.2 MB per core instead of 102.4 MB.

PSUM drain is split across two engines so neither becomes the bottleneck:
per 2048-col chunk, ScalarE does sqrt(dist^2)->f16 on cols [0:1024) and
VectorE copies raw dist^2->f16 on [1024:2048) (the host applies np.sqrt to
those fixed column ranges after the gather - DVE has no sqrt).

Per core: 32 b-tiles of 128 rows; per b-tile 3 chunks of 2048 + tail of 106;
one contiguous 1.6 MB store per b-tile (chunked on b-tile 0 to start the
store pipeline early).
"""

import numpy as np
from contextlib import ExitStack

import concourse.bass as bass
import concourse.bacc as bacc
import concourse.tile as tile
from concourse import mybir
from concourse.bass_utils import run_bass_kernel_spmd

B, D, K = 4096, 64, 50000
NCORES = 8
KS = K // NCORES  # 6250 columns per core
P = 128
JT = 512          # matmul free-dim tile (one PSUM bank of fp32)
DC = D + 4        # 68-row contraction: x | 1,1 | x2_hi,x2_lo
CHUNK = 4 * JT    # 2048: one 4-bank PSUM tile
SPLIT = 1024      # cols [0:SPLIT) of each chunk -> ScalarE sqrt,
                  # [SPLIT:CHUNK) -> VectorE raw-dist^2 copy (host sqrts)

F32 = mybir.dt.float32
F16 = mybir.dt.float16


def build_nc(b=B, ks=KS):
    nbt = b // P
    nc = bacc.Bacc("TRN2", target_bir_lowering=False, debug=False)
    xst = nc.dram_tensor("xst", [DC, b], F16, kind="ExternalInput").ap()
    wst = nc.dram_tensor("wst", [DC, ks], F16, kind="ExternalInput").ap()
    out = nc.dram_tensor("out", [b, ks], F16, kind="ExternalOutput").ap()

    chunks = [(c0, min(CHUNK, ks - c0)) for c0 in range(0, ks, CHUNK)]

    with tile.TileContext(nc) as tc:
        with ExitStack() as ctx:
            singles = ctx.enter_context(tc.tile_pool(name="singles", bufs=1))
            outp = ctx.enter_context(tc.tile_pool(name="outp", bufs=2))
            psum = ctx.enter_context(tc.tile_pool(name="psum", bufs=2, space="PSUM"))

            wst_sb = singles.tile([DC, ks], F16)
            xst_sb = singles.tile([DC, b], F16)
            # Load order = criticality: first j-tile weights + first b-tile x
            # gate the first matmul; the bulk follows and overlaps compute.
            nc.sync.dma_start(out=wst_sb[:, 0:JT], in_=wst[:, 0:JT])
            nc.sync.dma_start(out=xst_sb[:, 0:P], in_=xst[:, 0:P])
            nc.sync.dma_start(out=wst_sb[:, JT:CHUNK], in_=wst[:, JT:CHUNK])
            nc.sync.dma_start(out=xst_sb[:, P:b], in_=xst[:, P:b])
            nc.sync.dma_start(out=wst_sb[:, CHUNK:ks], in_=wst[:, CHUNK:ks])

            for ib in range(nbt):
                ot = outp.tile([P, ks], F16)
                lhs = xst_sb[:, ib * P:(ib + 1) * P]
                for ic, (c0, cn) in enumerate(chunks):
                    pt = psum.tile([P, CHUNK], F32)
                    for jj in range(0, cn, JT):
                        jn = min(JT, cn - jj)
                        nc.tensor.matmul(
                            pt[:, jj:jj + jn],
                            lhs,
                            wst_sb[:, c0 + jj:c0 + jj + jn],
                            start=True,
                            stop=True,
                        )
                    if cn == CHUNK:
                        nc.scalar.activation(
                            ot[:, c0:c0 + SPLIT],
                            pt[:, 0:SPLIT],
                            mybir.ActivationFunctionType.Sqrt,
                        )
                        nc.vector.tensor_copy(
                            ot[:, c0 + SPLIT:c0 + cn],
                            pt[:, SPLIT:cn],
                        )
                    else:
                        nc.scalar.activation(
                            ot[:, c0:c0 + cn],
                            pt[:, 0:cn],
                            mybir.ActivationFunctionType.Sqrt,
                        )
                    if ib == 0:
                        nc.sync.dma_start(
                            out=out[ib * P:(ib + 1) * P, c0:c0 + cn],
                            in_=ot[:, c0:c0 + cn],
                        )
                if ib > 0:
                    nc.sync.dma_start(out=out[ib * P:(ib + 1) * P, :], in_=ot)
    nc.compile()
    return nc


def prep_inputs(x, weight):
    """Host-side prep: fp16 operand matrices with hi/lo splits for w2/x2."""
    x = np.ascontiguousarray(x, dtype=np.float32)
    weight = np.ascontiguousarray(weight, dtype=np.float32)
    b, d = x.shape
    k = weight.shape[1]
    x2 = (x.astype(np.float64) ** 2).sum(axis=1).astype(np.float32)
    w2 = (weight.astype(np.float64) ** 2).sum(axis=0).astype(np.float32)

    x2_hi = x2.astype(np.float16)
    x2_lo = (x2 - x2_hi.astype(np.float32)).astype(np.float16)
    w2_hi = w2.astype(np.float16)
    w2_lo = (w2 - w2_hi.astype(np.float32)).astype(np.float16)

    xst = np.empty((DC, b), dtype=np.float16)
    xst[:d] = (-2.0 * x).T.astype(np.float16)
    xst[d] = 1.0
    xst[d + 1] = 1.0
    xst[d + 2] = x2_hi
    xst[d + 3] = x2_lo
    wst = np.empty((DC, k), dtype=np.float16)
    wst[:d] = weight.astype(np.float16)
    wst[d] = w2_hi
    wst[d + 1] = w2_lo
    wst[d + 2] = 1.0
    wst[d + 3] = 1.0
    return xst, wst


_nc_cache = {}


def _get_nc():
    if "nc" not in _nc_cache:
        _nc_cache["nc"] = build_nc()
    return _nc_cache["nc"]


def make_in_maps(x, weight, ks=KS):
    xst, wst = prep_inputs(x, weight)
    return [
        {"xst": xst,
         "wst": np.ascontiguousarray(wst[:, i * ks:(i + 1) * ks])}
        for i in range(NCORES)
    ]


def kernel(x, weight):
    nc = _get_nc()
    in_maps = make_in_maps(x, weight)
    res = run_bass_kernel_spmd(nc, in_maps, core_ids=list(range(NCORES)))
    full = np.concatenate(
        [res.results[i]["out"] for i in range(NCORES)], axis=1
    )  # f16 [B, K]
    out = full.astype(np.float32)
    # DVE-drained column ranges hold raw dist^2 -> sqrt on host.
    for i in range(NCORES):
        base = i * KS
        for c0 in range(0, KS - CHUNK + 1, CHUNK):  # full chunks only
            lo, hi = base + c0 + SPLIT, base + c0 + CHUNK
            np.sqrt(out[:, lo:hi], out=out[:, lo:hi])
    return out


# revision 5
# speedup vs baseline: 1.6578x; 1.1414x over previous
"""Euclidean distance (cdist) kernel for Trainium2, 8 NeuronCores.

out[b, j] = || x[b, :] - weight[:, j] ||_2   for x [4096, 64], weight [64, 50000].

Sharding (per hint): K = 50000 split into 8 slabs of 6250, one per core
(tensor-parallel over prototypes); x replicated; no cross-core reduction.

Math: dist^2 = x2[b] + w2[j] - 2*x@w, in ONE fp8e4m3 DoubleRow matmul per
output tile. DoubleRow streams 2 fp8 k-tiles per cycle (2x the fp16/fp32r
column rate, the dominant cost at B*K/128 streamed columns), and the full
128-partition x 2-k-tile contraction budget carries a Dekker-style product
(a = -2x, split a ~= a_hi + a_lo, w ~= w_hi + w_lo, each level e4m3):

  rows   0- 63, k0:  a_hi | w_hi      rows  0-63, k1:  a_lo | w_hi
  rows  64-127, k0:  a_hi | w_lo      rows 64-68, k1:   1   | w2_lvl0-4
                                      rows 69-73, k1: x2_lvl0-4 | 1
  => PSUM = a.w (3 of 4 Dekker terms, ~9-bit product) + w2 + x2 = dist^2
  (w2/x2 carried as five e4m3 levels each, ~20 bits; measured max rel err
   vs fp64 reference: 9.7e-4, gate is 2e-2)

Output is stored as fp16 (adds <=4.9e-4 rel), halving store traffic:
51.2 MB per core. PSUM drain splits across two engines so neither
bottlenecks: per 2048-col chunk ScalarE does sqrt->f16 on [0:SPLIT) and
VectorE copies raw dist^2->f16 on [SPLIT:2048) (host sqrts those fixed
ranges after the gather - DVE has no sqrt op).

Per core: 32 b-tiles of 128 rows; per b-tile 3 chunks of 2048 + tail 106;
one contiguous 1.6 MB store per b-tile (chunked on b-tile 0).
"""

import numpy as np
from contextlib import ExitStack

import ml_dtypes
import concourse.bass as bass
import concourse.bacc as bacc
import concourse.tile as tile
from concourse import mybir
from concourse.bass_utils import run_bass_kernel_spmd

B, D, K = 4096, 64, 50000
NCORES = 8
KS = K // NCORES  # 6250 columns per core
P = 128
JT = 512          # matmul free-dim tile (one PSUM bank of fp32)
CHUNK = 4 * JT    # 2048: one 4-bank PSUM tile
SPLIT = 1184      # cols [0:SPLIT) of each chunk -> ScalarE sqrt,
                  # [SPLIT:CHUNK) -> VectorE raw-dist^2 copy (host sqrts)
NLV = 5           # e4m3 levels carrying w2/x2

F32 = mybir.dt.float32
F16 = mybir.dt.float16
F8 = mybir.dt.float8e4
E4 = ml_dtypes.float8_e4m3


def build_nc(b=B, ks=KS):
    nbt = b // P
    nc = bacc.Bacc("TRN2", target_bir_lowering=False, debug=False)
    xq = nc.dram_tensor("xq", [P, 2, b], F8, kind="ExternalInput").ap()
    wq = nc.dram_tensor("wq", [P, 2, ks], F8, kind="ExternalInput").ap()
    out = nc.dram_tensor("out", [b, ks], F16, kind="ExternalOutput").ap()

    chunks = [(c0, min(CHUNK, ks - c0)) for c0 in range(0, ks, CHUNK)]

    with tile.TileContext(nc) as tc:
        with ExitStack() as ctx:
            singles = ctx.enter_context(tc.tile_pool(name="singles", bufs=1))
            outp = ctx.enter_context(tc.tile_pool(name="outp", bufs=3))
            psum = ctx.enter_context(tc.tile_pool(name="psum", bufs=2, space="PSUM"))

            wq_sb = singles.tile([P, 2, ks], F8)
            xq_sb = singles.tile([P, 2, b], F8)
            # Load order = criticality: first j-tile weights + first b-tile x
            # gate the first matmul; the bulk follows and overlaps compute.
            nc.sync.dma_start(out=wq_sb[:, :, 0:JT], in_=wq[:, :, 0:JT])
            nc.sync.dma_start(out=xq_sb[:, :, 0:P], in_=xq[:, :, 0:P])
            nc.sync.dma_start(out=wq_sb[:, :, JT:CHUNK], in_=wq[:, :, JT:CHUNK])
            nc.sync.dma_start(out=xq_sb[:, :, P:b], in_=xq[:, :, P:b])
            nc.sync.dma_start(out=wq_sb[:, :, CHUNK:ks], in_=wq[:, :, CHUNK:ks])

            for ib in range(nbt):
                ot = outp.tile([P, ks], F16)
                lhs = xq_sb[:, :, ib * P:(ib + 1) * P]
                for ic, (c0, cn) in enumerate(chunks):
                    pt = psum.tile([P, CHUNK], F32)
                    for jj in range(0, cn, JT):
                        jn = min(JT, cn - jj)
                        nc.tensor.matmul(
                            pt[:, jj:jj + jn],
                            lhs,
                            wq_sb[:, :, c0 + jj:c0 + jj + jn],
                            start=True,
                            stop=True,
                            perf_mode=mybir.MatmulPerfMode.DoubleRow,
                        )
                    if cn == CHUNK:
                        nc.scalar.activation(
                            ot[:, c0:c0 + SPLIT],
                            pt[:, 0:SPLIT],
                            mybir.ActivationFunctionType.Sqrt,
                        )
                        nc.vector.tensor_copy(
                            ot[:, c0 + SPLIT:c0 + cn],
                            pt[:, SPLIT:cn],
                        )
                    else:
                        nc.scalar.activation(
                            ot[:, c0:c0 + cn],
                            pt[:, 0:cn],
                            mybir.ActivationFunctionType.Sqrt,
                        )
                    if ib == 0:
                        nc.sync.dma_start(
                            out=out[ib * P:(ib + 1) * P, c0:c0 + cn],
                            in_=ot[:, c0:c0 + cn],
                        )
                if ib > 0:
                    nc.sync.dma_start(out=out[ib * P:(ib + 1) * P, :], in_=ot)
    nc.compile()
    return nc


def _f8(a):
    return a.astype(E4).astype(np.float32)


def _levels(v, n):
    """Greedy e4m3 decomposition v ~= sum(levels)."""
    out = []
    r = np.asarray(v, np.float32).copy()
    for _ in range(n):
        h = _f8(r)
        out.append(h)
        r = (r - h).astype(np.float32)
    return out


def prep_inputs(x, weight):
    """Host-side prep: fp8 DoubleRow operand tensors."""
    x = np.ascontiguousarray(x, dtype=np.float32)
    weight = np.ascontiguousarray(weight, dtype=np.float32)
    b, d = x.shape
    k = weight.shape[1]
    x2 = (x.astype(np.float64) ** 2).sum(axis=1).astype(np.float32)
    w2 = (weight.astype(np.float64) ** 2).sum(axis=0).astype(np.float32)

    a = (-2.0 * x).astype(np.float32)
    a_hi = _f8(a)
    a_lo = _f8((a - a_hi).astype(np.float32))
    w_hi = _f8(weight)
    w_lo = _f8((weight - w_hi).astype(np.float32))
    w2_lv = _levels(w2, NLV)
    x2_lv = _levels(x2, NLV)

    xq = np.zeros((P, 2, b), dtype=E4)
    xq[:d, 0] = a_hi.T.astype(E4)
    xq[:d, 1] = a_lo.T.astype(E4)
    xq[d:2 * d, 0] = a_hi.T.astype(E4)
    xq[d:d + NLV, 1] = 1.0
    for i in range(NLV):
        xq[d + NLV + i, 1] = x2_lv[i].astype(E4)

    wq = np.zeros((P, 2, k), dtype=E4)
    wq[:d, 0] = w_hi.astype(E4)
    wq[:d, 1] = w_hi.astype(E4)
    wq[d:2 * d, 0] = w_lo.astype(E4)
    for i in range(NLV):
        wq[d + i, 1] = w2_lv[i].astype(E4)
    wq[d + NLV:d + 2 * NLV, 1] = 1.0
    return xq, wq


_nc_cache = {}


def _get_nc():
    if "nc" not in _nc_cache:
        _nc_cache["nc"] = build_nc()
    return _nc_cache["nc"]


def make_in_maps(x, weight, ks=KS):
    xq, wq = prep_inputs(x, weight)
    return [
        {"xq": xq,
         "wq": np.ascontiguousarray(wq[:, :, i * ks:(i + 1) * ks])}
        for i in range(NCORES)
    ]


def kernel(x, weight):
    nc = _get_nc()
    in_maps = make_in_maps(x, weight)
    res = run_bass_kernel_spmd(nc, in_maps, core_ids=list(range(NCORES)))
    full = np.concatenate(
        [res.results[i]["out"] for i in range(NCORES)], axis=1
    )  # f16 [B, K]
    out = full.astype(np.float32)
    # DVE-drained column ranges hold raw dist^2 -> sqrt on host.
    for i in range(NCORES):
        base = i * KS
        for c0 in range(0, KS - CHUNK + 1, CHUNK):  # full chunks only
            lo, hi = base + c0 + SPLIT, base + c0 + CHUNK
            np.sqrt(out[:, lo:hi], out=out[:, lo:hi])
    return out


# revision 11
# speedup vs baseline: 1.7022x; 1.0268x over previous
"""Euclidean distance (cdist) kernel for Trainium2, 8 NeuronCores.

out[b, j] = || x[b, :] - weight[:, j] ||_2   for x [4096, 64], weight [64, 50000].

Sharding (per hint): K = 50000 split into 8 slabs of 6250, one per core
(tensor-parallel over prototypes); x replicated; no cross-core reduction.

Math: dist^2 = x2[b] + w2[j] - 2*x@w, in ONE fp8e4m3 DoubleRow matmul per
output tile. DoubleRow streams 2 fp8 k-tiles per cycle (2x the fp16/fp32r
column rate, the dominant cost at B*K/128 streamed columns), and the full
128-partition x 2-k-tile contraction budget carries a Dekker-style product
(a = -2x, split a ~= a_hi + a_lo, w ~= w_hi + w_lo, each level e4m3):

  rows   0- 63, k0:  a_hi | w_hi      rows  0-63, k1:  a_lo | w_hi
  rows  64-127, k0:  a_hi | w_lo      rows 64-68, k1:   1   | w2_lvl0-4
                                      rows 69-73, k1: x2_lvl0-4 | 1
  => PSUM = a.w (3 of 4 Dekker terms, ~9-bit product) + w2 + x2 = dist^2
  (w2/x2 carried as five e4m3 levels each, ~20 bits; measured max rel err
   vs fp64 reference: 9.7e-4, gate is 2e-2)

Output is stored as fp16 (adds <=4.9e-4 rel), halving store traffic:
51.2 MB per core. PSUM drain splits across two engines so neither
bottlenecks: per 2048-col chunk ScalarE does sqrt->f16 on [0:SPLIT) and
VectorE copies raw dist^2->f16 on [SPLIT:2048) (host sqrts those fixed
ranges after the gather - DVE has no sqrt op).

Per core: 32 b-tiles of 128 rows; per b-tile 3 chunks of 2048 + tail 106;
one contiguous 1.6 MB store per b-tile (chunked on b-tile 0).
"""

import numpy as np
from contextlib import ExitStack

import ml_dtypes
import concourse.bass as bass
import concourse.bacc as bacc
import concourse.tile as tile
from concourse import mybir
from concourse.bass_utils import run_bass_kernel_spmd

B, D, K = 4096, 64, 50000
NCORES = 8
KS = K // NCORES  # 6250 columns per core
P = 128
JT = 512          # matmul free-dim tile (one PSUM bank of fp32)
CHUNK = 4 * JT    # 2048: one 4-bank PSUM tile
SPLIT = 1088      # cols [0:SPLIT) of each chunk -> ScalarE sqrt,
                  # [SPLIT:CHUNK) -> VectorE raw-dist^2 copy (host sqrts)
NLV = 5           # e4m3 levels carrying w2/x2

F32 = mybir.dt.float32
F16 = mybir.dt.float16
F8 = mybir.dt.float8e4
E4 = ml_dtypes.float8_e4m3


def build_nc(b=B, ks=KS):
    nbt = b // P
    nc = bacc.Bacc("TRN2", target_bir_lowering=False, debug=False)
    xq = nc.dram_tensor("xq", [P, 2, b], F8, kind="ExternalInput").ap()
    wq = nc.dram_tensor("wq", [P, 2, ks], F8, kind="ExternalInput").ap()
    out = nc.dram_tensor("out", [b, ks], F16, kind="ExternalOutput").ap()

    chunks = [(c0, min(CHUNK, ks - c0)) for c0 in range(0, ks, CHUNK)]

    with tile.TileContext(nc) as tc:
        with ExitStack() as ctx:
            singles = ctx.enter_context(tc.tile_pool(name="singles", bufs=1))
            outp = ctx.enter_context(tc.tile_pool(name="outp", bufs=4))
            psum = ctx.enter_context(tc.tile_pool(name="psum", bufs=2, space="PSUM"))

            wq_sb = singles.tile([P, 2, ks], F8)
            xq_sb = singles.tile([P, 2, b], F8)
            # Load order = criticality: first j-tile weights + first b-tile x
            # gate the first matmul; the bulk follows and overlaps compute.
            nc.sync.dma_start(out=wq_sb[:, :, 0:JT], in_=wq[:, :, 0:JT])
            nc.sync.dma_start(out=xq_sb[:, :, 0:P], in_=xq[:, :, 0:P])
            nc.sync.dma_start(out=wq_sb[:, :, JT:CHUNK], in_=wq[:, :, JT:CHUNK])
            nc.sync.dma_start(out=xq_sb[:, :, P:b], in_=xq[:, :, P:b])
            nc.sync.dma_start(out=wq_sb[:, :, CHUNK:ks], in_=wq[:, :, CHUNK:ks])

            for ib in range(nbt):
                ot = outp.tile([P, ks], F16)
                lhs = xq_sb[:, :, ib * P:(ib + 1) * P]
                for ic, (c0, cn) in enumerate(chunks):
                    pt = psum.tile([P, CHUNK], F32)
                    for jj in range(0, cn, JT):
                        jn = min(JT, cn - jj)
                        nc.tensor.matmul(
                            pt[:, jj:jj + jn],
                            lhs,
                            wq_sb[:, :, c0 + jj:c0 + jj + jn],
                            start=True,
                            stop=True,
                            perf_mode=mybir.MatmulPerfMode.DoubleRow,
                        )
                    if cn == CHUNK:
                        nc.scalar.activation(
                            ot[:, c0:c0 + SPLIT],
                            pt[:, 0:SPLIT],
                            mybir.ActivationFunctionType.Sqrt,
                        )
                        nc.vector.tensor_copy(
                            ot[:, c0 + SPLIT:c0 + cn],
                            pt[:, SPLIT:cn],
                        )
                    else:
                        nc.scalar.activation(
                            ot[:, c0:c0 + cn],
                            pt[:, 0:cn],
                            mybir.ActivationFunctionType.Sqrt,
                        )
                    if ib == 0:
                        nc.sync.dma_start(
                            out=out[ib * P:(ib + 1) * P, c0:c0 + cn],
                            in_=ot[:, c0:c0 + cn],
                        )
                    elif ic == 1:
                        # first half-row store streams while chunk 2 drains
                        nc.sync.dma_start(
                            out=out[ib * P:(ib + 1) * P, 0:2 * CHUNK],
                            in_=ot[:, 0:2 * CHUNK],
                        )
                if ib > 0:
                    nc.sync.dma_start(
                        out=out[ib * P:(ib + 1) * P, 2 * CHUNK:ks],
                        in_=ot[:, 2 * CHUNK:ks],
                    )
    nc.compile()
    return nc


def _f8(a):
    return a.astype(E4).astype(np.float32)


def _levels(v, n):
    """Greedy e4m3 decomposition v ~= sum(levels)."""
    out = []
    r = np.asarray(v, np.float32).copy()
    for _ in range(n):
        h = _f8(r)
        out.append(h)
        r = (r - h).astype(np.float32)
    return out


def prep_inputs(x, weight):
    """Host-side prep: fp8 DoubleRow operand tensors."""
    x = np.ascontiguousarray(x, dtype=np.float32)
    weight = np.ascontiguousarray(weight, dtype=np.float32)
    b, d = x.shape
    k = weight.shape[1]
    x2 = (x.astype(np.float64) ** 2).sum(axis=1).astype(np.float32)
    w2 = (weight.astype(np.float64) ** 2).sum(axis=0).astype(np.float32)

    a = (-2.0 * x).astype(np.float32)
    a_hi = _f8(a)
    a_lo = _f8((a - a_hi).astype(np.float32))
    w_hi = _f8(weight)
    w_lo = _f8((weight - w_hi).astype(np.float32))
    w2_lv = _levels(w2, NLV)
    x2_lv = _levels(x2, NLV)

    xq = np.zeros((P, 2, b), dtype=E4)
    xq[:d, 0] = a_hi.T.astype(E4)
    xq[:d, 1] = a_lo.T.astype(E4)
    xq[d:2 * d, 0] = a_hi.T.astype(E4)
    xq[d:d + NLV, 1] = 1.0
    for i in range(NLV):
        xq[d + NLV + i, 1] = x2_lv[i].astype(E4)

    wq = np.zeros((P, 2, k), dtype=E4)
    wq[:d, 0] = w_hi.astype(E4)
    wq[:d, 1] = w_hi.astype(E4)
    wq[d:2 * d, 0] = w_lo.astype(E4)
    for i in range(NLV):
        wq[d + i, 1] = w2_lv[i].astype(E4)
    wq[d + NLV:d + 2 * NLV, 1] = 1.0
    return xq, wq


_nc_cache = {}


def _get_nc():
    if "nc" not in _nc_cache:
        _nc_cache["nc"] = build_nc()
    return _nc_cache["nc"]


def make_in_maps(x, weight, ks=KS):
    xq, wq = prep_inputs(x, weight)
    return [
        {"xq": xq,
         "wq": np.ascontiguousarray(wq[:, :, i * ks:(i + 1) * ks])}
        for i in range(NCORES)
    ]


def kernel(x, weight):
    nc = _get_nc()
    in_maps = make_in_maps(x, weight)
    res = run_bass_kernel_spmd(nc, in_maps, core_ids=list(range(NCORES)))
    full = np.concatenate(
        [res.results[i]["out"] for i in range(NCORES)], axis=1
    )  # f16 [B, K]
    out = full.astype(np.float32)
    # DVE-drained column ranges hold raw dist^2 -> sqrt on host.
    for i in range(NCORES):
        base = i * KS
        for c0 in range(0, KS - CHUNK + 1, CHUNK):  # full chunks only
            lo, hi = base + c0 + SPLIT, base + c0 + CHUNK
            np.sqrt(out[:, lo:hi], out=out[:, lo:hi])
    return out


# revision 15
# speedup vs baseline: 2.2811x; 1.3401x over previous
"""Euclidean distance (cdist) kernel for Trainium2, 8 NeuronCores.

out[b, j] = || x[b, :] - weight[:, j] ||_2   for x [4096, 64], weight [64, 50000].

Sharding (per hint): K = 50000 split into 8 slabs of 6250, one per core
(tensor-parallel over prototypes); x replicated; no cross-core reduction.

Math: dist^2 = x2[b] + w2[j] - 2*x@w, in ONE fp8e4m3 DoubleRow matmul per
output tile. DoubleRow streams 2 fp8 k-tiles per cycle (2x the fp16/fp32r
column rate, the dominant cost at B*K/128 streamed columns), and the full
128-partition x 2-k-tile contraction budget carries a Dekker-style product
(a = -2x, split a ~= a_hi + a_lo, w ~= w_hi + w_lo, each level e4m3):

  rows   0- 63, k0:  a_hi | w_hi      rows  0-63, k1:  a_lo | w_hi
  rows  64-127, k0:  a_hi | w_lo      rows 64-68, k1:   1   | w2_lvl0-4
                                      rows 69-73, k1: x2_lvl0-4 | 1
  => PSUM = a.w (3 of 4 Dekker terms, ~9-bit product) + w2 + x2 = dist^2
  (w2/x2 carried as five e4m3 levels each, ~20 bits; measured max rel err
   vs fp64 reference: 9.7e-4, gate is 2e-2)

Output is stored as fp16 (adds <=4.9e-4 rel), halving store traffic:
51.2 MB per core. PSUM drain splits across two engines so neither
bottlenecks: per 2048-col chunk ScalarE does sqrt->f16 on [0:SPLIT) and
VectorE copies raw dist^2->f16 on [SPLIT:2048) (host sqrts those fixed
ranges after the gather - DVE has no sqrt op).

Per core: 32 b-tiles of 128 rows; per b-tile 3 chunks of 2048 + tail 106;
one contiguous 1.6 MB store per b-tile (chunked on b-tile 0).
"""

import numpy as np
from contextlib import ExitStack

import ml_dtypes
import concourse.bass as bass
import concourse.bacc as bacc
import concourse.tile as tile
from concourse import mybir
from concourse.bass_utils import run_bass_kernel_spmd

B, D, K = 4096, 64, 50000
NCORES = 8
KS = K // NCORES  # 6250 columns per core
P = 128
JT = 512          # matmul free-dim tile (one PSUM bank of fp32)
CHUNK = 2 * JT    # 1024: one 2-bank PSUM tile; 4 in flight hides drain latency
NLV = 5           # e4m3 levels carrying w2/x2
# Whole-chunk drains alternate engines: even chunks (+tail) -> ScalarE sqrt,
# odd chunks -> VectorE raw-dist^2 copy (host sqrts those ranges).

F32 = mybir.dt.float32
F16 = mybir.dt.float16
F8 = mybir.dt.float8e4
E4 = ml_dtypes.float8_e4m3


def build_nc(b=B, ks=KS):
    nbt = b // P
    nc = bacc.Bacc("TRN2", target_bir_lowering=False, debug=False)
    xq = nc.dram_tensor("xq", [P, 2, b], F8, kind="ExternalInput").ap()
    wq = nc.dram_tensor("wq", [P, 2, ks], F8, kind="ExternalInput").ap()
    out = nc.dram_tensor("out", [b, ks], F16, kind="ExternalOutput").ap()

    chunks = [(c0, min(CHUNK, ks - c0)) for c0 in range(0, ks, CHUNK)]

    with tile.TileContext(nc) as tc:
        with ExitStack() as ctx:
            singles = ctx.enter_context(tc.tile_pool(name="singles", bufs=1))
            outp = ctx.enter_context(tc.tile_pool(name="outp", bufs=4))
            psum = ctx.enter_context(tc.tile_pool(name="psum", bufs=4, space="PSUM"))

            wq_sb = singles.tile([P, 2, ks], F8)
            xq_sb = singles.tile([P, 2, b], F8)
            # Load order = criticality: first j-tile weights + first b-tile x
            # gate the first matmul; the bulk follows and overlaps compute.
            nc.sync.dma_start(out=wq_sb[:, :, 0:JT], in_=wq[:, :, 0:JT])
            nc.sync.dma_start(out=xq_sb[:, :, 0:P], in_=xq[:, :, 0:P])
            nc.sync.dma_start(out=wq_sb[:, :, JT:CHUNK], in_=wq[:, :, JT:CHUNK])
            nc.sync.dma_start(out=xq_sb[:, :, P:b], in_=xq[:, :, P:b])
            nc.sync.dma_start(out=wq_sb[:, :, CHUNK:ks], in_=wq[:, :, CHUNK:ks])

            for ib in range(nbt):
                ot = outp.tile([P, ks], F16)
                lhs = xq_sb[:, :, ib * P:(ib + 1) * P]
                for ic, (c0, cn) in enumerate(chunks):
                    pt = psum.tile([P, CHUNK], F32)
                    for jj in range(0, cn, JT):
                        jn = min(JT, cn - jj)
                        nc.tensor.matmul(
                            pt[:, jj:jj + jn],
                            lhs,
                            wq_sb[:, :, c0 + jj:c0 + jj + jn],
                            start=True,
                            stop=True,
                            perf_mode=mybir.MatmulPerfMode.DoubleRow,
                        )
                    if cn < CHUNK or ic % 2 == 0:
                        nc.scalar.activation(
                            ot[:, c0:c0 + cn],
                            pt[:, 0:cn],
                            mybir.ActivationFunctionType.Sqrt,
                        )
                    else:
                        nc.vector.tensor_copy(
                            ot[:, c0:c0 + cn],
                            pt[:, 0:cn],
                        )
                    if ib == 0:
                        nc.sync.dma_start(
                            out=out[ib * P:(ib + 1) * P, c0:c0 + cn],
                            in_=ot[:, c0:c0 + cn],
                        )
                    elif ic == 3:
                        # first half-row store streams while later chunks drain
                        nc.sync.dma_start(
                            out=out[ib * P:(ib + 1) * P, 0:4 * CHUNK],
                            in_=ot[:, 0:4 * CHUNK],
                        )
                if ib > 0:
                    nc.sync.dma_start(
                        out=out[ib * P:(ib + 1) * P, 4 * CHUNK:ks],
                        in_=ot[:, 4 * CHUNK:ks],
                    )
    nc.compile()
    return nc


def _f8(a):
    return a.astype(E4).astype(np.float32)


def _levels(v, n):
    """Greedy e4m3 decomposition v ~= sum(levels)."""
    out = []
    r = np.asarray(v, np.float32).copy()
    for _ in range(n):
        h = _f8(r)
        out.append(h)
        r = (r - h).astype(np.float32)
    return out


def prep_inputs(x, weight):
    """Host-side prep: fp8 DoubleRow operand tensors."""
    x = np.ascontiguousarray(x, dtype=np.float32)
    weight = np.ascontiguousarray(weight, dtype=np.float32)
    b, d = x.shape
    k = weight.shape[1]
    x2 = (x.astype(np.float64) ** 2).sum(axis=1).astype(np.float32)
    w2 = (weight.astype(np.float64) ** 2).sum(axis=0).astype(np.float32)

    a = (-2.0 * x).astype(np.float32)
    a_hi = _f8(a)
    a_lo = _f8((a - a_hi).astype(np.float32))
    w_hi = _f8(weight)
    w_lo = _f8((weight - w_hi).astype(np.float32))
    w2_lv = _levels(w2, NLV)
    x2_lv = _levels(x2, NLV)

    xq = np.zeros((P, 2, b), dtype=E4)
    xq[:d, 0] = a_hi.T.astype(E4)
    xq[:d, 1] = a_lo.T.astype(E4)
    xq[d:2 * d, 0] = a_hi.T.astype(E4)
    xq[d:d + NLV, 1] = 1.0
    for i in range(NLV):
        xq[d + NLV + i, 1] = x2_lv[i].astype(E4)

    wq = np.zeros((P, 2, k), dtype=E4)
    wq[:d, 0] = w_hi.astype(E4)
    wq[:d, 1] = w_hi.astype(E4)
    wq[d:2 * d, 0] = w_lo.astype(E4)
    for i in range(NLV):
        wq[d + i, 1] = w2_lv[i].astype(E4)
    wq[d + NLV:d + 2 * NLV, 1] = 1.0
    return xq, wq


_nc_cache = {}


def _get_nc():
    if "nc" not in _nc_cache:
        _nc_cache["nc"] = build_nc()
    return _nc_cache["nc"]


def make_in_maps(x, weight, ks=KS):
    xq, wq = prep_inputs(x, weight)
    return [
        {"xq": xq,
         "wq": np.ascontiguousarray(wq[:, :, i * ks:(i + 1) * ks])}
        for i in range(NCORES)
    ]


def kernel(x, weight):
    nc = _get_nc()
    in_maps = make_in_maps(x, weight)
    res = run_bass_kernel_spmd(nc, in_maps, core_ids=list(range(NCORES)))
    full = np.concatenate(
        [res.results[i]["out"] for i in range(NCORES)], axis=1
    )  # f16 [B, K]
    out = full.astype(np.float32)
    # DVE-drained (odd full) chunks hold raw dist^2 -> sqrt on host.
    for i in range(NCORES):
        base = i * KS
        for c0 in range(CHUNK, KS - CHUNK + 1, 2 * CHUNK):
            np.sqrt(out[:, base + c0:base + c0 + CHUNK],
                    out=out[:, base + c0:base + c0 + CHUNK])
    return out


# revision 17
# speedup vs baseline: 2.6732x; 1.1719x over previous
"""Euclidean distance (cdist) kernel for Trainium2, 8 NeuronCores.

out[b, j] = || x[b, :] - weight[:, j] ||_2   for x [4096, 64], weight [64, 50000].

Sharding (per hint): K = 50000 split into 8 slabs of 6250, one per core
(tensor-parallel over prototypes); x replicated; no cross-core reduction.

Math: dist^2 = x2[b] + w2[j] - 2*x@w, in ONE fp8e4m3 DoubleRow matmul per
output tile. DoubleRow streams 2 fp8 k-tiles per cycle (2x the fp16/fp32r
column rate, the dominant cost at B*K/128 streamed columns), and the full
128-partition x 2-k-tile contraction budget carries a Dekker-style product
(a = -2x, split a ~= a_hi + a_lo, w ~= w_hi + w_lo, each level e4m3):

  rows   0- 63, k0:  a_hi | w_hi      rows  0-63, k1:  a_lo | w_hi
  rows  64-127, k0:  a_hi | w_lo      rows 64-68, k1:   1   | w2_lvl0-4
                                      rows 69-73, k1: x2_lvl0-4 | 1
  => PSUM = a.w (3 of 4 Dekker terms, ~9-bit product) + w2 + x2 = dist^2
  (w2/x2 carried as five e4m3 levels each, ~20 bits; measured max rel err
   vs fp64 reference: 9.7e-4, gate is 2e-2)

Output is stored as fp16 (adds <=4.9e-4 rel), halving store traffic:
51.2 MB per core. PSUM drain splits across two engines so neither
bottlenecks: per 2048-col chunk ScalarE does sqrt->f16 on [0:SPLIT) and
VectorE copies raw dist^2->f16 on [SPLIT:2048) (host sqrts those fixed
ranges after the gather - DVE has no sqrt op).

Per core: 32 b-tiles of 128 rows; per b-tile 3 chunks of 2048 + tail 106;
one contiguous 1.6 MB store per b-tile (chunked on b-tile 0).
"""

import numpy as np
from contextlib import ExitStack

import ml_dtypes
import concourse.bass as bass
import concourse.bacc as bacc
import concourse.tile as tile
from concourse import mybir
from concourse.bass_utils import run_bass_kernel_spmd

B, D, K = 4096, 64, 50000
NCORES = 8
KS = K // NCORES  # 6250 columns per core
P = 128
JT = 512          # matmul free-dim tile (one PSUM bank of fp32)
CHUNK = 2 * JT    # 1024: one 2-bank PSUM tile; 4 in flight hides drain latency
NLV = 5           # e4m3 levels carrying w2/x2
# Whole-chunk drains alternate engines: even chunks (+tail) -> ScalarE sqrt,
# odd chunks -> VectorE raw-dist^2 copy (host sqrts those ranges).

F32 = mybir.dt.float32
F16 = mybir.dt.float16
F8 = mybir.dt.float8e4
E4 = ml_dtypes.float8_e4m3


def build_nc(b=B, ks=KS):
    nbt = b // P
    nc = bacc.Bacc("TRN2", target_bir_lowering=False, debug=False)
    xq = nc.dram_tensor("xq", [P, 2, b], F8, kind="ExternalInput").ap()
    wq = nc.dram_tensor("wq", [P, 2, ks], F8, kind="ExternalInput").ap()
    out = nc.dram_tensor("out", [b, ks], F16, kind="ExternalOutput").ap()

    chunks = [(c0, min(CHUNK, ks - c0)) for c0 in range(0, ks, CHUNK)]

    with tile.TileContext(nc) as tc:
        with ExitStack() as ctx:
            singles = ctx.enter_context(tc.tile_pool(name="singles", bufs=1))
            outp = ctx.enter_context(tc.tile_pool(name="outp", bufs=6))
            psum = ctx.enter_context(tc.tile_pool(name="psum", bufs=4, space="PSUM"))

            wq_sb = singles.tile([P, 2, ks], F8)
            xq_sb = singles.tile([P, 2, b], F8)
            # Load order = criticality: first j-tile weights + first b-tile x
            # gate the first matmul; the bulk follows and overlaps compute.
            nc.sync.dma_start(out=wq_sb[:, :, 0:JT], in_=wq[:, :, 0:JT])
            nc.sync.dma_start(out=xq_sb[:, :, 0:P], in_=xq[:, :, 0:P])
            nc.sync.dma_start(out=wq_sb[:, :, JT:CHUNK], in_=wq[:, :, JT:CHUNK])
            nc.sync.dma_start(out=xq_sb[:, :, P:b], in_=xq[:, :, P:b])
            nc.sync.dma_start(out=wq_sb[:, :, CHUNK:ks], in_=wq[:, :, CHUNK:ks])

            for ib in range(nbt):
                ot = outp.tile([P, ks], F16)
                lhs = xq_sb[:, :, ib * P:(ib + 1) * P]
                for ic, (c0, cn) in enumerate(chunks):
                    pt = psum.tile([P, CHUNK], F32)
                    for jj in range(0, cn, JT):
                        jn = min(JT, cn - jj)
                        nc.tensor.matmul(
                            pt[:, jj:jj + jn],
                            lhs,
                            wq_sb[:, :, c0 + jj:c0 + jj + jn],
                            start=True,
                            stop=True,
                            perf_mode=mybir.MatmulPerfMode.DoubleRow,
                        )
                    if cn < CHUNK or ic % 2 == 0:
                        nc.scalar.activation(
                            ot[:, c0:c0 + cn],
                            pt[:, 0:cn],
                            mybir.ActivationFunctionType.Sqrt,
                        )
                    else:
                        nc.vector.tensor_copy(
                            ot[:, c0:c0 + cn],
                            pt[:, 0:cn],
                        )
                    if ib == 0:
                        # pairwise stores: [0:2048]@ic1, [2048:4096]@ic3,
                        # [4096:6250]@tail - keeps lines >= 2KB
                        if ic in (1, 3):
                            nc.sync.dma_start(
                                out=out[0:P, c0 + cn - 2 * CHUNK:c0 + cn],
                                in_=ot[:, c0 + cn - 2 * CHUNK:c0 + cn],
                            )
                        elif ic == len(chunks) - 1:
                            nc.sync.dma_start(
                                out=out[0:P, 4 * CHUNK:ks],
                                in_=ot[:, 4 * CHUNK:ks],
                            )
                    elif ic == 3:
                        # first half-row store streams while later chunks drain
                        nc.sync.dma_start(
                            out=out[ib * P:(ib + 1) * P, 0:4 * CHUNK],
                            in_=ot[:, 0:4 * CHUNK],
                        )
                if ib > 0:
                    nc.sync.dma_start(
                        out=out[ib * P:(ib + 1) * P, 4 * CHUNK:ks],
                        in_=ot[:, 4 * CHUNK:ks],
                    )
    nc.compile()
    return nc


def _f8(a):
    return a.astype(E4).astype(np.float32)


def _levels(v, n):
    """Greedy e4m3 decomposition v ~= sum(levels)."""
    out = []
    r = np.asarray(v, np.float32).copy()
    for _ in range(n):
        h = _f8(r)
        out.append(h)
        r = (r - h).astype(np.float32)
    return out


def prep_inputs(x, weight):
    """Host-side prep: fp8 DoubleRow operand tensors."""
    x = np.ascontiguousarray(x, dtype=np.float32)
    weight = np.ascontiguousarray(weight, dtype=np.float32)
    b, d = x.shape
    k = weight.shape[1]
    x2 = (x.astype(np.float64) ** 2).sum(axis=1).astype(np.float32)
    w2 = (weight.astype(np.float64) ** 2).sum(axis=0).astype(np.float32)

    a = (-2.0 * x).astype(np.float32)
    a_hi = _f8(a)
    a_lo = _f8((a - a_hi).astype(np.float32))
    w_hi = _f8(weight)
    w_lo = _f8((weight - w_hi).astype(np.float32))
    w2_lv = _levels(w2, NLV)
    x2_lv = _levels(x2, NLV)

    xq = np.zeros((P, 2, b), dtype=E4)
    xq[:d, 0] = a_hi.T.astype(E4)
    xq[:d, 1] = a_lo.T.astype(E4)
    xq[d:2 * d, 0] = a_hi.T.astype(E4)
    xq[d:d + NLV, 1] = 1.0
    for i in range(NLV):
        xq[d + NLV + i, 1] = x2_lv[i].astype(E4)

    wq = np.zeros((P, 2, k), dtype=E4)
    wq[:d, 0] = w_hi.astype(E4)
    wq[:d, 1] = w_hi.astype(E4)
    wq[d:2 * d, 0] = w_lo.astype(E4)
    for i in range(NLV):
        wq[d + i, 1] = w2_lv[i].astype(E4)
    wq[d + NLV:d + 2 * NLV, 1] = 1.0
    return xq, wq


_nc_cache = {}


def _get_nc():
    if "nc" not in _nc_cache:
        _nc_cache["nc"] = build_nc()
    return _nc_cache["nc"]


def make_in_maps(x, weight, ks=KS):
    xq, wq = prep_inputs(x, weight)
    return [
        {"xq": xq,
         "wq": np.ascontiguousarray(wq[:, :, i * ks:(i + 1) * ks])}
        for i in range(NCORES)
    ]


def kernel(x, weight):
    nc = _get_nc()
    in_maps = make_in_maps(x, weight)
    res = run_bass_kernel_spmd(nc, in_maps, core_ids=list(range(NCORES)))
    full = np.concatenate(
        [res.results[i]["out"] for i in range(NCORES)], axis=1
    )  # f16 [B, K]
    out = full.astype(np.float32)
    # DVE-drained (odd full) chunks hold raw dist^2 -> sqrt on host.
    for i in range(NCORES):
        base = i * KS
        for c0 in range(CHUNK, KS - CHUNK + 1, 2 * CHUNK):
            np.sqrt(out[:, base + c0:base + c0 + CHUNK],
                    out=out[:, base + c0:base + c0 + CHUNK])
    return out
